# revision 49
# baseline (speedup 1.0000x reference)
"""Trainium2 Bass kernel for a soft-MoE (MANN) block.

Reference math (per token b):
    g  = elu(x_gate @ g1_w.T + g1_b); g = elu(g @ g2_w.T + g2_b)
    ew = softmax(g @ g3_w.T + g3_b)                      # [B, K=8]
    h1 = elu(sum_k ew_k * (x_main @ W1_k.T) + ew @ b1)   # [B, 1024]
    h2 = elu(sum_k ew_k * (h1 @ W2_k.T) + ew @ b2)       # [B, 1024]
    y  =     sum_k ew_k * (h2 @ W3_k.T) + ew @ b3        # [B, 640]

Strategy: data-parallel over 8 NeuronCores (128 batch rows per core),
expert weights replicated and streamed from HBM with W1/W2 in fp8-e3m4
(exact per-layer scale folded into the on-chip ew broadcast) and W3 in
fp16; fp32 PSUM accumulation throughout. All trunk layers run
weight-stationary so layer outputs come out feature-major and feed the
next layer with no transposes; the final y is stored feature-major and
transposed on the host. The batch is processed in two 64-token halves
so vector/activation ELU+scale work on one half overlaps PE matmuls on
the other. All gating parameters arrive in one packed DMA so the
weight stream starts immediately.
"""

import sys

sys.path.insert(0, "/opt/trn_rl_repo")

from contextlib import ExitStack

import numpy as np
import ml_dtypes

import concourse.bass as bass
from concourse import bacc
import concourse.tile as tile
from concourse import mybir
from concourse.bass_utils import run_bass_kernel_spmd
from concourse.masks import make_identity

F32 = mybir.dt.float32
BF16 = mybir.dt.bfloat16
FP16 = mybir.dt.float16
E3M4 = mybir.dt.float8e3
AF = mybir.ActivationFunctionType
OP = mybir.AluOpType

B = 1024
X_MAIN, X_GATE, Y_DIM = 480, 128, 640
HID, GHID, K = 1024, 64, 8
NCORES = 8
BS = B // NCORES  # 128 batch rows per core
HB = BS // 2  # half-batch for DVE/PE pipelining

E3M4_MAX = 15.5

# packed gating-parameter column layout (one [64, GP_COLS] f32 DMA; 64
# partitions halve the DMA descriptor-generation latency on the critical path)
GP_XG = 0          # [0:256]   x_gate.T as [64, 2, BS]
GP_G1W = 256       # [256:384] g1_w.T as [64, 2, GHID]
GP_G2W = 384       # [384:448] g2_w.T
GP_G3W = 448       # [448:456] g3_w.T
GP_G1B = 456       # col 456   g1_b
GP_G2B = 457       # col 457   g2_b adjusted
GP_G3B = 458       # [458:466] g3_b adjusted (partition 0)
GP_COLS = 466

# trunk layer configs: (partition size, #i-tiles, O, weight dtype)
# weights stream in [P, ITC, O] tiles so DMA granularity stays fine-grained
ITC = 4
LCFG = (
    (120, 4, HID, E3M4),
    (128, 8, HID, E3M4),
    (128, 8, Y_DIM, FP16),
)


def _build_program(with_bias: tuple[bool, bool, bool],
                   rs1: float, rs2: float) -> bass.Bass:
    nc = bacc.Bacc()

    # ---- DRAM parameters (host supplies exactly these layouts) ----
    gp_ext = nc.declare_dram_parameter("gp", [GHID, GP_COLS], F32, isOutput=False)
    msk_ext = nc.declare_dram_parameter("msk", [K, K * BS], BF16, isOutput=False)
    xm_ext = nc.declare_dram_parameter("xm", [120, 4, BS], BF16, isOutput=False)
    w_ext = []
    c_ext = []
    for li, (P, IT, O, wdt) in enumerate(LCFG):
        w_ext.append(
            nc.declare_dram_parameter(f"w{li + 1}", [K, P, IT, O], wdt, isOutput=False)
        )
        if with_bias[li]:
            c_ext.append(
                nc.declare_dram_parameter(f"c{li + 1}", [K, O], BF16, isOutput=False)
            )
        else:
            c_ext.append(None)
    y_ext = nc.declare_dram_parameter(
        "y", [2, 128, Y_DIM // 128, HB], F32, isOutput=True
    )

    with tile.TileContext(nc) as tc, ExitStack() as ctx:
        const = ctx.enter_context(tc.tile_pool(name="const", bufs=1))
        gat = ctx.enter_context(tc.tile_pool(name="gat", bufs=1))
        spsum = ctx.enter_context(tc.tile_pool(name="spsum", bufs=2, space="PSUM"))
        bpsum = ctx.enter_context(tc.tile_pool(name="bpsum", bufs=1, space="PSUM"))
        zpsum = ctx.enter_context(tc.tile_pool(name="zpsum", bufs=4, space="PSUM"))
        xpool = ctx.enter_context(tc.tile_pool(name="xpool", bufs=1))
        xkp = ctx.enter_context(tc.tile_pool(name="xkp", bufs=1))
        hscr = ctx.enter_context(tc.tile_pool(name="hscr", bufs=2))
        wp = [
            ctx.enter_context(tc.tile_pool(name="w1p", bufs=4)),
            ctx.enter_context(tc.tile_pool(name="w2p", bufs=12)),
            ctx.enter_context(tc.tile_pool(name="w3p", bufs=14)),
        ]

        ident = const.tile([128, 128], F32)
        make_identity(nc, ident)
        ones = const.tile([1, BS], F32)
        nc.vector.memset(ones, 1.0)
        # touch the activation engine immediately so its function-table load
        # (1.3us) runs during the DMA head instead of gating the first exp
        dumo = const.tile([1, 2], F32, name="dumo")
        nc.scalar.activation(dumo, ones[:, 0:2], AF.Exp)

        # spin the tensor engine so its clock is ramped before gating starts
        warm = spsum.tile([128, 128], F32, tag="g", name="warm")
        for _ in range(14):
            nc.tensor.transpose(warm, ident, ident)

        # ---------------- gating (fp32) ----------------
        gp_sb = gat.tile([GHID, GP_COLS], F32)
        nc.sync.dma_start(gp_sb, gp_ext[:])
        mask = const.tile([K, K * BS], BF16)
        nc.sync.dma_start(mask, msk_ext[:])
        x1_sb = xpool.tile([120, 4, BS], BF16, tag="x1")
        nc.sync.dma_start(x1_sb, xm_ext[:])

        xg_sb = gp_sb[:, GP_XG : GP_XG + 2 * BS]
        g1w_sb = gp_sb[:, GP_G1W : GP_G1W + 2 * GHID]
        g2w_sb = gp_sb[:, GP_G2W : GP_G2W + GHID]
        g3w_sb = gp_sb[:, GP_G3W : GP_G3W + K]
        g1b_sb = gp_sb[:, GP_G1B : GP_G1B + 1]
        g2b_sb = gp_sb[:, GP_G2B : GP_G2B + 1]
        g3b_sb = gp_sb[0:1, GP_G3B : GP_G3B + K]

        def gate_elup(zp, bias_ap, name):
            # elu(w) + 1 = relu(w) + min(exp(w), 1) with w = z + bias.
            # Gating logits are O(1) here so exp(w) cannot overflow.
            e = gat.tile([GHID, BS], F32, tag=f"e_{name}")
            nc.scalar.activation(e, zp, AF.Exp, bias=bias_ap)
            r = gat.tile([GHID, BS], F32, tag=f"r_{name}")
            nc.vector.tensor_scalar(r, zp, bias_ap, 0.0, OP.add, OP.max)
            hp = gat.tile([GHID, BS], F32, tag=f"hp_{name}")
            nc.vector.scalar_tensor_tensor(hp, e, 1.0, r, OP.min, OP.add)
            return hp

        zg1 = spsum.tile([GHID, BS], F32, tag="g")
        for d in range(2):
            nc.tensor.matmul(
                zg1,
                lhsT=gp_sb[:, GP_G1W + d * GHID : GP_G1W + (d + 1) * GHID],
                rhs=gp_sb[:, GP_XG + d * BS : GP_XG + (d + 1) * BS],
                start=(d == 0), stop=(d == 1),
            )
        h1p = gate_elup(zg1, g1b_sb, "g1")

        zg2 = spsum.tile([GHID, BS], F32, tag="g")
        nc.tensor.matmul(zg2, lhsT=g2w_sb, rhs=h1p, start=True, stop=True)
        h2p = gate_elup(zg2, g2b_sb, "g2")

        # logits in [b, k] layout
        zg3 = spsum.tile([BS, K], F32, tag="g")
        nc.tensor.matmul(zg3, lhsT=h2p, rhs=g3w_sb, start=True, stop=False)
        nc.tensor.matmul(zg3, lhsT=ones, rhs=g3b_sb, start=False, stop=True)

        # softmax along free dim (K); logits here are O(1) so exp without
        # the usual max-subtraction is safe
        e3 = gat.tile([BS, K], F32)
        ssum = gat.tile([BS, 1], F32)
        nc.scalar.activation(e3, zg3, AF.Exp, accum_out=ssum[:, 0:1])
        rcp = gat.tile([BS, 1], F32)
        nc.vector.reciprocal(rcp, ssum)
        ewT = gat.tile([BS, K], F32)  # [b, k]
        nc.vector.tensor_scalar_mul(ewT, e3, rcp[:, 0:1])

        # fast lane: the first token-half of ew reaches expert 0's scaled
        # input through half-width ops before the full-width pipeline runs
        ewps0 = spsum.tile([K, HB], F32, tag="g", name="ewps0")
        nc.tensor.transpose(ewps0, ewT[0:HB, :], ident[0:HB, 0:HB])
        ew_sb0 = gat.tile([K, HB], BF16, name="ewsb0")
        nc.vector.tensor_copy(out=ew_sb0, in_=ewps0)

        # ew on partitions 0..K-1: [K, BS]
        ewps = spsum.tile([K, BS], F32, tag="g")
        nc.tensor.transpose(ewps, ewT, ident)
        ew_sb = gat.tile([K, BS], BF16)
        nc.vector.tensor_copy(out=ew_sb, in_=ewps)

        # broadcast each ew row to all 128 partitions via one-hot matmuls;
        # two PSUM tiles so early experts' consumers wait on fewer writers
        ebps = [bpsum.tile([128, 4, BS], F32, name=f"ebp{i}") for i in range(2)]
        nc.tensor.matmul(
            ebps[0][:, 0, 0:HB], lhsT=mask[:, 0:BS], rhs=ew_sb0,
            start=True, stop=True, skip_group_check=True,
        )
        nc.tensor.matmul(
            ebps[0][:, 0, HB:BS], lhsT=mask[:, 0:BS], rhs=ew_sb[:, HB:BS],
            start=False, stop=True, skip_group_check=True,
        )
        for k in range(1, K):
            nc.tensor.matmul(
                ebps[k // 4][:, k % 4, :],
                lhsT=mask[:, k * BS : (k + 1) * BS], rhs=ew_sb,
                start=True, stop=True,
            )

        # per-layer scaled ew broadcasts (bf16): L1,L2 carry 1/s_l, L3 raw.
        # ewb1/xk1 are emitted first, per-expert, so L1 starts sooner; the
        # L2/L3 variants are built afterwards (they are not latency-critical)
        ewb = [
            gat.tile([128, K, BS], BF16, tag=f"ewb{li}", name=f"ewb{li}")
            for li in range(3)
        ]

        # ---------------- trunk ----------------
        xks = [
            xkp.tile([LCFG[li][0], K, LCFG[li][1], BS], BF16, tag=f"xk{li}",
                     name=f"xk{li}")
            for li in range(3)
        ]
        for k in range(K):
            if k == 0:
                halves = ((slice(0, HB), HB), (slice(HB, BS), HB))
                for hsl, hn in halves:
                    nc.vector.tensor_scalar(
                        ewb[0][:, 0, hsl], ebps[0][:, 0, hsl], rs1, None,
                        OP.mult,
                    )
                    nc.vector.tensor_tensor(
                        xks[0][:, 0, :, hsl],
                        x1_sb[:, :, hsl],
                        ewb[0][:120, 0, None, hsl].to_broadcast((120, 4, hn)),
                        OP.mult,
                    )
                continue
            nc.vector.tensor_scalar(
                ewb[0][:, k, :], ebps[k // 4][:, k % 4, :], rs1, None, OP.mult,
            )
            nc.vector.tensor_tensor(
                xks[0][:, k],
                x1_sb,
                ewb[0][:120, k, None, :].to_broadcast((120, 4, BS)),
                OP.mult,
            )
        for i in range(2):
            nc.vector.tensor_scalar(
                ewb[1][:, 4 * i : 4 * i + 4], ebps[i], rs2, None, OP.mult
            )
            nc.vector.tensor_copy(
                out=ewb[2][:, 4 * i : 4 * i + 4], in_=ebps[i]
            )

        x_sb = x1_sb
        for li, (P, IT, O, wdt) in enumerate(LCFG):
            last = li == 2
            OT = O // 128
            ND = IT // ITC  # weight dma tiles per expert
            xk = xks[li]
            if li > 0:
                # k-major emission to match the PE's consumption order; k=0
                # was already produced by the previous layer's ELU tail
                for k in range(1, K):
                    for h in range(2):
                        hs = slice(h * HB, (h + 1) * HB)
                        nc.vector.tensor_tensor(
                            xk[:, k, :, hs],
                            x_sb[:, :, hs],
                            ewb[li][:P, k, None, hs].to_broadcast((P, IT, HB)),
                            OP.mult,
                        )
            if not last:
                nx_sb = xpool.tile([128, OT, BS], BF16, tag=f"x{li + 2}")
            if c_ext[li] is not None:
                cl_sb = gat.tile([K, O], BF16, tag=f"bias{li}")
                nc.sync.dma_start(cl_sb, c_ext[li][:])

            zps = []
            for h in range(2):
                zp = zpsum.tile([128, OT, HB], F32, tag="z", name=f"zp{li}_{h}")
                if c_ext[li] is not None:
                    for ot in range(OT):
                        nc.tensor.matmul(
                            zp[:, ot, :],
                            lhsT=cl_sb[:, ot * 128 : (ot + 1) * 128],
                            rhs=ew_sb[:, h * HB : (h + 1) * HB],
                            start=(ot == 0), stop=False,
                            skip_group_check=True,
                        )
                zps.append(zp)

            for k in range(K):
                tiles = []
                for d in range(ND):
                    w_sb = wp[li].tile(
                        [P, ITC, O], wdt, tag=f"w{li}", name=f"w{li}_{k}_{d}"
                    )
                    nc.sync.dma_start(
                        w_sb, w_ext[li][k][:, d * ITC : (d + 1) * ITC]
                    )
                    tiles.append(w_sb)
                if k < K - 1:
                    order = [(d, h) for d in range(ND) for h in range(2)]
                else:
                    # close the h0 accumulation early so the ELU / y writeout
                    # of the first half overlaps the second half's matmuls
                    order = [(d, h) for h in range(2) for d in range(ND)]
                for d, h in order:
                    hs = slice(h * HB, (h + 1) * HB)
                    for itl in range(ITC):
                        it = d * ITC + itl
                        for ot in range(OT):
                            # one accumulation group per PSUM bank: only
                            # the first write opens (and zeroes) the bank
                            nc.tensor.matmul(
                                zps[h][:, ot, :],
                                lhsT=tiles[d][:, itl, ot * 128 : (ot + 1) * 128],
                                rhs=xk[:, k, it, hs],
                                start=(k == 0 and it == 0 and ot == 0
                                       and c_ext[li] is None),
                                stop=(k == K - 1 and it == IT - 1
                                      and ot == OT - 1),
                                skip_group_check=True,
                            )

            if last:
                for h in range(2):
                    zp = zps[h]
                    y_sb = xpool.tile([128, OT, HB], F32, tag=f"y{h}")
                    nc.vector.tensor_copy(out=y_sb, in_=zp)
                    nc.sync.dma_start(y_ext[h], y_sb)
            else:
                # elu(z) = min(exp(z),1) - 1 + max(z,0); trunk z is O(0.1) so
                # exp cannot overflow, and ACT/DVE run in parallel. Quartered
                # in the exact order the next layer's first expert consumes,
                # each quarter immediately followed by that expert's scaled
                # input so the next layer's matmuls start as soon as possible.
                P2 = LCFG[li + 1][0]
                for og in range(OT // ITC):
                    og_s = slice(og * ITC, (og + 1) * ITC)
                    for h in range(2):
                        hs = slice(h * HB, (h + 1) * HB)
                        zp = zps[h]
                        e = hscr.tile([128, ITC, HB], F32, tag="he",
                                      name=f"he{li}_{og}_{h}")
                        nc.scalar.activation(e, zp[:, og_s], AF.Exp)
                        r = hscr.tile([128, ITC, HB], F32, tag="hr",
                                      name=f"hr{li}_{og}_{h}")
                        nc.vector.tensor_scalar(r, zp[:, og_s], 0.0, -1.0,
                                                OP.max, OP.add)
                        nc.vector.scalar_tensor_tensor(
                            nx_sb[:, og_s, hs], e, 1.0, r, OP.min, OP.add
                        )
                        nc.vector.tensor_tensor(
                            xks[li + 1][:, 0, og_s, hs],
                            nx_sb[:, og_s, hs],
                            ewb[li + 1][:P2, 0, None, hs].to_broadcast(
                                (P2, ITC, HB)
                            ),
                            OP.mult,
                        )
                x_sb = nx_sb

    nc.compile()
    return nc


_PROG_CACHE: dict = {}


def _get_program(with_bias, s1, s2):
    key = (tuple(with_bias), float(s1), float(s2))
    if key not in _PROG_CACHE:
        _PROG_CACHE[key] = _build_program(
            tuple(with_bias), float(1.0 / s1), float(1.0 / s2)
        )
    return _PROG_CACHE[key]


def _onehot_mask():
    m = np.zeros((K, K * BS), ml_dtypes.bfloat16)
    for k in range(K):
        m[k, k * BS : (k + 1) * BS] = 1.0
    return m


def _prep_w(W, P, IT, np_dt, scale):
    # [K, O, I] -> [K, P, IT, O] with element [k,p,it,o] = W[k,o,it*P+p]
    Kk, O, I = W.shape
    Wt = W.transpose(0, 2, 1).reshape(Kk, IT, P, O).transpose(0, 2, 1, 3)
    if scale != 1.0:
        Wt = Wt * np.float32(scale)
    return np.ascontiguousarray(Wt.astype(np_dt))


def kernel(
    x_main, x_gate, g1_w, g1_b, g2_w, g2_b, g3_w, g3_b,
    W1, b1, W2, b2, W3, b3,
):
    x_main = np.asarray(x_main, np.float32)
    x_gate = np.asarray(x_gate, np.float32)
    g1_w = np.asarray(g1_w, np.float32)
    g1_b = np.asarray(g1_b, np.float32)
    g2_w = np.asarray(g2_w, np.float32)
    g2_b = np.asarray(g2_b, np.float32)
    g3_w = np.asarray(g3_w, np.float32)
    g3_b = np.asarray(g3_b, np.float32)
    W1 = np.asarray(W1, np.float32)
    b1 = np.asarray(b1, np.float32)
    W2 = np.asarray(W2, np.float32)
    b2 = np.asarray(b2, np.float32)
    W3 = np.asarray(W3, np.float32)
    b3 = np.asarray(b3, np.float32)

    with_bias = (bool(b1.any()), bool(b2.any()), bool(b3.any()))
    s1 = E3M4_MAX * 0.9999 / max(np.abs(W1).max(), 1e-30)
    s2 = E3M4_MAX * 0.9999 / max(np.abs(W2).max(), 1e-30)
    nc = _get_program(with_bias, s1, s2)

    # packed gating parameters, shared across cores except xg
    gp_base = np.zeros((GHID, GP_COLS), np.float32)
    gp_base[:, GP_G1W : GP_G1W + 2 * GHID] = (
        g1_w.T.reshape(2, GHID, GHID).transpose(1, 0, 2).reshape(GHID, 2 * GHID)
    )
    gp_base[:, GP_G2W : GP_G2W + GHID] = g2_w.T
    gp_base[:, GP_G3W : GP_G3W + K] = g3_w.T
    gp_base[:, GP_G1B] = g1_b
    gp_base[:, GP_G2B] = g2_b - g2_w.sum(1)
    gp_base[0, GP_G3B : GP_G3B + K] = g3_b - g3_w.sum(1)

    shared = {
        "msk": _onehot_mask(),
        "w1": _prep_w(W1, 120, 4, ml_dtypes.float8_e3m4, s1),
        "w2": _prep_w(W2, 128, 8, ml_dtypes.float8_e3m4, s2),
        "w3": _prep_w(W3, 128, 8, np.float16, 1.0),
    }
    for name, b, flag in (("c1", b1, with_bias[0]), ("c2", b2, with_bias[1]),
                          ("c3", b3, with_bias[2])):
        if flag:
            shared[name] = np.ascontiguousarray(b.astype(ml_dtypes.bfloat16))

    in_maps = []
    for s in range(NCORES):
        xm_s = x_main[s * BS : (s + 1) * BS].T  # [480, BS]
        xm_s = np.ascontiguousarray(
            xm_s.reshape(4, 120, BS).transpose(1, 0, 2).astype(ml_dtypes.bfloat16)
        )  # [120, 4, BS]
        gp = gp_base.copy()
        gp[:, GP_XG : GP_XG + 2 * BS] = (
            x_gate[s * BS : (s + 1) * BS].T
            .reshape(2, GHID, BS).transpose(1, 0, 2).reshape(GHID, 2 * BS)
        )
        in_maps.append({**shared, "gp": gp, "xm": xm_s})

    res = run_bass_kernel_spmd(nc, in_maps, list(range(NCORES))).results
    outs = []
    for s in range(NCORES):
        y_s = res[s]["y"]  # [2, 128, 5, HB] half/feature-major
        outs.append(
            np.ascontiguousarray(
                y_s.transpose(0, 3, 2, 1).reshape(BS, Y_DIM)
            )
        )
    return np.concatenate(outs, axis=0)


# revision 50
# speedup vs baseline: 1.0142x; 1.0142x over previous
"""Trainium2 Bass kernel for a soft-MoE (MANN) block.

Reference math (per token b):
    g  = elu(x_gate @ g1_w.T + g1_b); g = elu(g @ g2_w.T + g2_b)
    ew = softmax(g @ g3_w.T + g3_b)                      # [B, K=8]
    h1 = elu(sum_k ew_k * (x_main @ W1_k.T) + ew @ b1)   # [B, 1024]
    h2 = elu(sum_k ew_k * (h1 @ W2_k.T) + ew @ b2)       # [B, 1024]
    y  =     sum_k ew_k * (h2 @ W3_k.T) + ew @ b3        # [B, 640]

Strategy: data-parallel over 8 NeuronCores (128 batch rows per core),
expert weights replicated and streamed from HBM with W1/W2 in fp8-e3m4
(exact per-layer scale folded into the on-chip ew broadcast) and W3 in
fp16; fp32 PSUM accumulation throughout. All trunk layers run
weight-stationary so layer outputs come out feature-major and feed the
next layer with no transposes; the final y is stored feature-major and
transposed on the host. The batch is processed in two 64-token halves
so vector/activation ELU+scale work on one half overlaps PE matmuls on
the other. All gating parameters arrive in one packed DMA so the
weight stream starts immediately.
"""

import sys

sys.path.insert(0, "/opt/trn_rl_repo")

from contextlib import ExitStack

import numpy as np
import ml_dtypes

import concourse.bass as bass
from concourse import bacc
import concourse.tile as tile
from concourse import mybir
from concourse.bass_utils import run_bass_kernel_spmd
from concourse.masks import make_identity

F32 = mybir.dt.float32
BF16 = mybir.dt.bfloat16
FP16 = mybir.dt.float16
E3M4 = mybir.dt.float8e3
AF = mybir.ActivationFunctionType
OP = mybir.AluOpType

B = 1024
X_MAIN, X_GATE, Y_DIM = 480, 128, 640
HID, GHID, K = 1024, 64, 8
NCORES = 8
BS = B // NCORES  # 128 batch rows per core
HB = BS // 2  # half-batch for DVE/PE pipelining

E3M4_MAX = 15.5

# packed gating-parameter column layout (one [64, GP_COLS] f32 DMA; 64
# partitions halve the DMA descriptor-generation latency on the critical path)
GP_XG = 0          # [0:256]   x_gate.T as [64, 2, BS]
GP_G1W = 256       # [256:384] g1_w.T as [64, 2, GHID]
GP_G2W = 384       # [384:448] g2_w.T
GP_G3W = 448       # [448:456] g3_w.T
GP_G1B = 456       # col 456   g1_b
GP_G2B = 457       # col 457   g2_b adjusted
GP_G3B = 458       # [458:466] g3_b adjusted (partition 0)
GP_COLS = 466

# trunk layer configs: (partition size, #i-tiles, O, weight dtype)
# weights stream in [P, ITC, O] tiles so DMA granularity stays fine-grained
ITC = 4
LCFG = (
    (120, 4, HID, E3M4),
    (128, 8, HID, E3M4),
    (128, 8, Y_DIM, FP16),
)


def _build_program(with_bias: tuple[bool, bool, bool],
                   rs1: float, rs2: float) -> bass.Bass:
    nc = bacc.Bacc()

    # ---- DRAM parameters (host supplies exactly these layouts) ----
    gp_ext = nc.declare_dram_parameter("gp", [GHID, GP_COLS], F32, isOutput=False)
    msk_ext = nc.declare_dram_parameter("msk", [K, K * BS], BF16, isOutput=False)
    xm_ext = nc.declare_dram_parameter("xm", [120, 4, BS], BF16, isOutput=False)
    w_ext = []
    c_ext = []
    for li, (P, IT, O, wdt) in enumerate(LCFG):
        w_ext.append(
            nc.declare_dram_parameter(f"w{li + 1}", [K, P, IT, O], wdt, isOutput=False)
        )
        if with_bias[li]:
            c_ext.append(
                nc.declare_dram_parameter(f"c{li + 1}", [K, O], BF16, isOutput=False)
            )
        else:
            c_ext.append(None)
    y_ext = nc.declare_dram_parameter(
        "y", [2, 128, Y_DIM // 128, HB], F32, isOutput=True
    )

    with tile.TileContext(nc) as tc, ExitStack() as ctx:
        const = ctx.enter_context(tc.tile_pool(name="const", bufs=1))
        gat = ctx.enter_context(tc.tile_pool(name="gat", bufs=1))
        spsum = ctx.enter_context(tc.tile_pool(name="spsum", bufs=2, space="PSUM"))
        bpsum = ctx.enter_context(tc.tile_pool(name="bpsum", bufs=1, space="PSUM"))
        zpsum = ctx.enter_context(tc.tile_pool(name="zpsum", bufs=4, space="PSUM"))
        xpool = ctx.enter_context(tc.tile_pool(name="xpool", bufs=1))
        xkp = ctx.enter_context(tc.tile_pool(name="xkp", bufs=1))
        hscr = ctx.enter_context(tc.tile_pool(name="hscr", bufs=3))
        wp = [
            ctx.enter_context(tc.tile_pool(name="w1p", bufs=4)),
            ctx.enter_context(tc.tile_pool(name="w2p", bufs=12)),
            ctx.enter_context(tc.tile_pool(name="w3p", bufs=14)),
        ]

        ident = const.tile([128, 128], F32)
        make_identity(nc, ident)
        ones = const.tile([1, BS], F32)
        nc.vector.memset(ones, 1.0)
        # touch the activation engine immediately so its function-table load
        # (1.3us) runs during the DMA head instead of gating the first exp
        dumo = const.tile([1, 2], F32, name="dumo")
        nc.scalar.activation(dumo, ones[:, 0:2], AF.Exp)

        # spin the tensor engine so its clock is ramped before gating starts
        warm = spsum.tile([128, 128], F32, tag="g", name="warm")
        for _ in range(4):
            nc.tensor.transpose(warm, ident, ident)

        # ---------------- gating (fp32) ----------------
        gp_sb = gat.tile([GHID, GP_COLS], F32)
        nc.sync.dma_start(gp_sb, gp_ext[:])
        mask = const.tile([K, K * BS], BF16)
        nc.sync.dma_start(mask, msk_ext[:])
        x1_sb = xpool.tile([120, 4, BS], BF16, tag="x1")
        nc.sync.dma_start(x1_sb, xm_ext[:])

        xg_sb = gp_sb[:, GP_XG : GP_XG + 2 * BS]
        g1w_sb = gp_sb[:, GP_G1W : GP_G1W + 2 * GHID]
        g2w_sb = gp_sb[:, GP_G2W : GP_G2W + GHID]
        g3w_sb = gp_sb[:, GP_G3W : GP_G3W + K]
        g1b_sb = gp_sb[:, GP_G1B : GP_G1B + 1]
        g2b_sb = gp_sb[:, GP_G2B : GP_G2B + 1]
        g3b_sb = gp_sb[0:1, GP_G3B : GP_G3B + K]

        def gate_elup(zp, bias_ap, name):
            # elu(w) + 1 = relu(w) + min(exp(w), 1) with w = z + bias.
            # Gating logits are O(1) here so exp(w) cannot overflow.
            e = gat.tile([GHID, BS], F32, tag=f"e_{name}")
            nc.scalar.activation(e, zp, AF.Exp, bias=bias_ap)
            r = gat.tile([GHID, BS], F32, tag=f"r_{name}")
            nc.vector.tensor_scalar(r, zp, bias_ap, 0.0, OP.add, OP.max)
            hp = gat.tile([GHID, BS], F32, tag=f"hp_{name}")
            nc.vector.scalar_tensor_tensor(hp, e, 1.0, r, OP.min, OP.add)
            return hp

        zg1 = spsum.tile([GHID, BS], F32, tag="g")
        for d in range(2):
            nc.tensor.matmul(
                zg1,
                lhsT=gp_sb[:, GP_G1W + d * GHID : GP_G1W + (d + 1) * GHID],
                rhs=gp_sb[:, GP_XG + d * BS : GP_XG + (d + 1) * BS],
                start=(d == 0), stop=(d == 1),
            )
        h1p = gate_elup(zg1, g1b_sb, "g1")

        zg2 = spsum.tile([GHID, BS], F32, tag="g")
        nc.tensor.matmul(zg2, lhsT=g2w_sb, rhs=h1p, start=True, stop=True)
        h2p = gate_elup(zg2, g2b_sb, "g2")

        # logits in [b, k] layout
        zg3 = spsum.tile([BS, K], F32, tag="g")
        nc.tensor.matmul(zg3, lhsT=h2p, rhs=g3w_sb, start=True, stop=False)
        nc.tensor.matmul(zg3, lhsT=ones, rhs=g3b_sb, start=False, stop=True)

        # softmax along free dim (K); logits here are O(1) so exp without
        # the usual max-subtraction is safe
        e3 = gat.tile([BS, K], F32)
        ssum = gat.tile([BS, 1], F32)
        nc.scalar.activation(e3, zg3, AF.Exp, accum_out=ssum[:, 0:1])
        rcp = gat.tile([BS, 1], F32)
        nc.vector.reciprocal(rcp, ssum)
        ewT = gat.tile([BS, K], F32)  # [b, k]
        nc.vector.tensor_scalar_mul(ewT, e3, rcp[:, 0:1])

        # fast lane: the first token-half of ew reaches expert 0's scaled
        # input through half-width ops before the full-width pipeline runs
        ewps0 = spsum.tile([K, HB], F32, tag="g", name="ewps0")
        nc.tensor.transpose(ewps0, ewT[0:HB, :], ident[0:HB, 0:HB])
        ew_sb0 = gat.tile([K, HB], BF16, name="ewsb0")
        nc.vector.tensor_copy(out=ew_sb0, in_=ewps0)

        # ew on partitions 0..K-1: [K, BS]
        ewps = spsum.tile([K, BS], F32, tag="g")
        nc.tensor.transpose(ewps, ewT, ident)
        ew_sb = gat.tile([K, BS], BF16)
        nc.vector.tensor_copy(out=ew_sb, in_=ewps)

        # broadcast each ew row to all 128 partitions via one-hot matmuls;
        # two PSUM tiles so early experts' consumers wait on fewer writers
        ebps = [bpsum.tile([128, 4, BS], F32, name=f"ebp{i}") for i in range(2)]
        nc.tensor.matmul(
            ebps[0][:, 0, 0:HB], lhsT=mask[:, 0:BS], rhs=ew_sb0,
            start=True, stop=True, skip_group_check=True,
        )
        nc.tensor.matmul(
            ebps[0][:, 0, HB:BS], lhsT=mask[:, 0:BS], rhs=ew_sb[:, HB:BS],
            start=False, stop=True, skip_group_check=True,
        )
        for k in range(1, K):
            nc.tensor.matmul(
                ebps[k // 4][:, k % 4, :],
                lhsT=mask[:, k * BS : (k + 1) * BS], rhs=ew_sb,
                start=True, stop=True,
            )

        # per-layer scaled ew broadcasts (bf16): L1,L2 carry 1/s_l, L3 raw.
        # ewb1/xk1 are emitted first, per-expert, so L1 starts sooner; the
        # L2/L3 variants are built afterwards (they are not latency-critical)
        ewb = [
            gat.tile([128, K, BS], BF16, tag=f"ewb{li}", name=f"ewb{li}")
            for li in range(3)
        ]

        # ---------------- trunk ----------------
        xks = [
            xkp.tile([LCFG[li][0], K, LCFG[li][1], BS], BF16, tag=f"xk{li}",
                     name=f"xk{li}")
            for li in range(3)
        ]
        for k in range(K):
            if k == 0:
                halves = ((slice(0, HB), HB), (slice(HB, BS), HB))
                for hsl, hn in halves:
                    nc.vector.tensor_scalar(
                        ewb[0][:, 0, hsl], ebps[0][:, 0, hsl], rs1, None,
                        OP.mult,
                    )
                    nc.vector.tensor_tensor(
                        xks[0][:, 0, :, hsl],
                        x1_sb[:, :, hsl],
                        ewb[0][:120, 0, None, hsl].to_broadcast((120, 4, hn)),
                        OP.mult,
                    )
                continue
            nc.vector.tensor_scalar(
                ewb[0][:, k, :], ebps[k // 4][:, k % 4, :], rs1, None, OP.mult,
            )
            nc.vector.tensor_tensor(
                xks[0][:, k],
                x1_sb,
                ewb[0][:120, k, None, :].to_broadcast((120, 4, BS)),
                OP.mult,
            )
        for i in range(2):
            nc.vector.tensor_scalar(
                ewb[1][:, 4 * i : 4 * i + 4], ebps[i], rs2, None, OP.mult
            )
            nc.vector.tensor_copy(
                out=ewb[2][:, 4 * i : 4 * i + 4], in_=ebps[i]
            )

        x_sb = x1_sb
        for li, (P, IT, O, wdt) in enumerate(LCFG):
            last = li == 2
            OT = O // 128
            ND = IT // ITC  # weight dma tiles per expert
            xk = xks[li]
            if li > 0:
                # k-major emission to match the PE's consumption order; k=0
                # was already produced by the previous layer's ELU tail
                for k in range(1, K):
                    for h in range(2):
                        hs = slice(h * HB, (h + 1) * HB)
                        nc.vector.tensor_tensor(
                            xk[:, k, :, hs],
                            x_sb[:, :, hs],
                            ewb[li][:P, k, None, hs].to_broadcast((P, IT, HB)),
                            OP.mult,
                        )
            if not last:
                nx_sb = xpool.tile([128, OT, BS], BF16, tag=f"x{li + 2}")
            if c_ext[li] is not None:
                cl_sb = gat.tile([K, O], BF16, tag=f"bias{li}")
                nc.sync.dma_start(cl_sb, c_ext[li][:])

            zps = []
            for h in range(2):
                zp = zpsum.tile([128, OT, HB], F32, tag="z", name=f"zp{li}_{h}")
                if c_ext[li] is not None:
                    for ot in range(OT):
                        nc.tensor.matmul(
                            zp[:, ot, :],
                            lhsT=cl_sb[:, ot * 128 : (ot + 1) * 128],
                            rhs=ew_sb[:, h * HB : (h + 1) * HB],
                            start=(ot == 0), stop=False,
                            skip_group_check=True,
                        )
                zps.append(zp)

            for k in range(K):
                tiles = []
                for d in range(ND):
                    w_sb = wp[li].tile(
                        [P, ITC, O], wdt, tag=f"w{li}", name=f"w{li}_{k}_{d}"
                    )
                    nc.sync.dma_start(
                        w_sb, w_ext[li][k][:, d * ITC : (d + 1) * ITC]
                    )
                    tiles.append(w_sb)
                if k < K - 1:
                    order = [(d, h) for d in range(ND) for h in range(2)]
                else:
                    # close the h0 accumulation early so the ELU / y writeout
                    # of the first half overlaps the second half's matmuls
                    order = [(d, h) for h in range(2) for d in range(ND)]
                for d, h in order:
                    hs = slice(h * HB, (h + 1) * HB)
                    for itl in range(ITC):
                        it = d * ITC + itl
                        for ot in range(OT):
                            # one accumulation group per PSUM bank: only
                            # the first write opens (and zeroes) the bank
                            nc.tensor.matmul(
                                zps[h][:, ot, :],
                                lhsT=tiles[d][:, itl, ot * 128 : (ot + 1) * 128],
                                rhs=xk[:, k, it, hs],
                                start=(k == 0 and it == 0 and ot == 0
                                       and c_ext[li] is None),
                                stop=(k == K - 1 and it == IT - 1
                                      and ot == OT - 1),
                                skip_group_check=True,
                            )

            if last:
                for h in range(2):
                    zp = zps[h]
                    y_sb = xpool.tile([128, OT, HB], F32, tag=f"y{h}")
                    nc.vector.tensor_copy(out=y_sb, in_=zp)
                    nc.sync.dma_start(y_ext[h], y_sb)
            else:
                # elu(z) = min(exp(z),1) - 1 + max(z,0); trunk z is O(0.1) so
                # exp cannot overflow, and ACT/DVE run in parallel. Quartered
                # in the exact order the next layer's first expert consumes,
                # each quarter immediately followed by that expert's scaled
                # input so the next layer's matmuls start as soon as possible.
                P2 = LCFG[li + 1][0]
                for og in range(OT // ITC):
                    og_s = slice(og * ITC, (og + 1) * ITC)
                    for h in range(2):
                        hs = slice(h * HB, (h + 1) * HB)
                        zp = zps[h]
                        e = hscr.tile([128, ITC, HB], F32, tag="he",
                                      name=f"he{li}_{og}_{h}")
                        nc.scalar.activation(e, zp[:, og_s], AF.Exp)
                        r = hscr.tile([128, ITC, HB], F32, tag="hr",
                                      name=f"hr{li}_{og}_{h}")
                        nc.vector.tensor_scalar(r, zp[:, og_s], 0.0, -1.0,
                                                OP.max, OP.add)
                        nc.vector.scalar_tensor_tensor(
                            nx_sb[:, og_s, hs], e, 1.0, r, OP.min, OP.add
                        )
                        nc.vector.tensor_tensor(
                            xks[li + 1][:, 0, og_s, hs],
                            nx_sb[:, og_s, hs],
                            ewb[li + 1][:P2, 0, None, hs].to_broadcast(
                                (P2, ITC, HB)
                            ),
                            OP.mult,
                        )
                x_sb = nx_sb

    nc.compile()
    return nc


_PROG_CACHE: dict = {}


def _get_program(with_bias, s1, s2):
    key = (tuple(with_bias), float(s1), float(s2))
    if key not in _PROG_CACHE:
        _PROG_CACHE[key] = _build_program(
            tuple(with_bias), float(1.0 / s1), float(1.0 / s2)
        )
    return _PROG_CACHE[key]


def _onehot_mask():
    m = np.zeros((K, K * BS), ml_dtypes.bfloat16)
    for k in range(K):
        m[k, k * BS : (k + 1) * BS] = 1.0
    return m


def _prep_w(W, P, IT, np_dt, scale):
    # [K, O, I] -> [K, P, IT, O] with element [k,p,it,o] = W[k,o,it*P+p]
    Kk, O, I = W.shape
    Wt = W.transpose(0, 2, 1).reshape(Kk, IT, P, O).transpose(0, 2, 1, 3)
    if scale != 1.0:
        Wt = Wt * np.float32(scale)
    return np.ascontiguousarray(Wt.astype(np_dt))


def kernel(
    x_main, x_gate, g1_w, g1_b, g2_w, g2_b, g3_w, g3_b,
    W1, b1, W2, b2, W3, b3,
):
    x_main = np.asarray(x_main, np.float32)
    x_gate = np.asarray(x_gate, np.float32)
    g1_w = np.asarray(g1_w, np.float32)
    g1_b = np.asarray(g1_b, np.float32)
    g2_w = np.asarray(g2_w, np.float32)
    g2_b = np.asarray(g2_b, np.float32)
    g3_w = np.asarray(g3_w, np.float32)
    g3_b = np.asarray(g3_b, np.float32)
    W1 = np.asarray(W1, np.float32)
    b1 = np.asarray(b1, np.float32)
    W2 = np.asarray(W2, np.float32)
    b2 = np.asarray(b2, np.float32)
    W3 = np.asarray(W3, np.float32)
    b3 = np.asarray(b3, np.float32)

    with_bias = (bool(b1.any()), bool(b2.any()), bool(b3.any()))
    s1 = E3M4_MAX * 0.9999 / max(np.abs(W1).max(), 1e-30)
    s2 = E3M4_MAX * 0.9999 / max(np.abs(W2).max(), 1e-30)
    nc = _get_program(with_bias, s1, s2)

    # packed gating parameters, shared across cores except xg
    gp_base = np.zeros((GHID, GP_COLS), np.float32)
    gp_base[:, GP_G1W : GP_G1W + 2 * GHID] = (
        g1_w.T.reshape(2, GHID, GHID).transpose(1, 0, 2).reshape(GHID, 2 * GHID)
    )
    gp_base[:, GP_G2W : GP_G2W + GHID] = g2_w.T
    gp_base[:, GP_G3W : GP_G3W + K] = g3_w.T
    gp_base[:, GP_G1B] = g1_b
    gp_base[:, GP_G2B] = g2_b - g2_w.sum(1)
    gp_base[0, GP_G3B : GP_G3B + K] = g3_b - g3_w.sum(1)

    shared = {
        "msk": _onehot_mask(),
        "w1": _prep_w(W1, 120, 4, ml_dtypes.float8_e3m4, s1),
        "w2": _prep_w(W2, 128, 8, ml_dtypes.float8_e3m4, s2),
        "w3": _prep_w(W3, 128, 8, np.float16, 1.0),
    }
    for name, b, flag in (("c1", b1, with_bias[0]), ("c2", b2, with_bias[1]),
                          ("c3", b3, with_bias[2])):
        if flag:
            shared[name] = np.ascontiguousarray(b.astype(ml_dtypes.bfloat16))

    in_maps = []
    for s in range(NCORES):
        xm_s = x_main[s * BS : (s + 1) * BS].T  # [480, BS]
        xm_s = np.ascontiguousarray(
            xm_s.reshape(4, 120, BS).transpose(1, 0, 2).astype(ml_dtypes.bfloat16)
        )  # [120, 4, BS]
        gp = gp_base.copy()
        gp[:, GP_XG : GP_XG + 2 * BS] = (
            x_gate[s * BS : (s + 1) * BS].T
            .reshape(2, GHID, BS).transpose(1, 0, 2).reshape(GHID, 2 * BS)
        )
        in_maps.append({**shared, "gp": gp, "xm": xm_s})

    res = run_bass_kernel_spmd(nc, in_maps, list(range(NCORES))).results
    outs = []
    for s in range(NCORES):
        y_s = res[s]["y"]  # [2, 128, 5, HB] half/feature-major
        outs.append(
            np.ascontiguousarray(
                y_s.transpose(0, 3, 2, 1).reshape(BS, Y_DIM)
            )
        )
    return np.concatenate(outs, axis=0)
